# revision 41
# baseline (speedup 1.0000x reference)
"""Trainium2 Bass kernel for nn_BehaviorPlant (Powderworld plant-growth step).

Data-parallel over batch: B=32 split across 8 NeuronCores (4 samples each).

Per sample (C=20 channels of 256x256 fp32):
  - plant_counts    = 3x3 ones-conv of world[PLANT]
  - wood_ice_counts = 3x3 ones-conv of world[ICE] + world[WOOD]
  - boolean masks a (grow plant), b (grow empty) from threshold logic
  - out[c] = world[c] except where a|b: out[c] = a*pv[c] + b*ev[c]

On-chip layout: each 256x256 plane lives in SBUF as [128, 512]
(partition p holds rows 2p and 2p+1; free = (row%2)*256 + col). The two
DRAM rows per partition are contiguous, so every DMA descriptor is 2 KiB
(vs 1 KiB for a row-per-partition layout) — halves descriptor overhead.
The conv's vertical pass runs on the TensorEngine via identity/shift
matrices (exact 0/1 weights, PSUM accumulation, strictly ascending add
order); the horizontal pass is shifted free-dim adds on the VectorEngine.

The per-channel blend value q = a*pv[c] + b*ev[c] is built from an ACT
scale (a*pv) plus a fused scale-add (b*ev + q1, alternating DVE stt /
ACT+POOL); a single DVE copy_predicated then overwrites the world tile
in place where a|b — kept pixels are never touched, and selected pixels
get exactly pv[c], ev[c], or fl(pv[c]+ev[c]), matching the reference
bit-for-bit (verified: 0/41943040 mismatches on hardware).

Cost-model span: 133.6 us/core vs a ~120 us HBM-traffic floor
(43 MB/core at ~360 GB/s); all compute engines have >=10% slack, so the
kernel is DMA-bound as the memory target_regime intends.
"""
import numpy as np

import concourse.tile as tile
from concourse import bacc, bass, mybir
from concourse.bass_utils import run_bass_kernel_spmd

# Powderworld element channel indices
EMPTY, WATER, WOOD, ICE, PLANT = 0, 3, 5, 6, 8

B, C, H, W = 32, 20, 256, 256
N_CORES = 8
S = B // N_CORES          # samples per core
P = 128                   # partitions
BLK = W                   # 256 columns per row-block
PL = 2 * BLK              # 512 = free size of one plane tile
HC = C // 2               # 10 channels per half-sample DMA

F32 = mybir.dt.float32
BF16 = mybir.dt.bfloat16

M_I, M_SD, M_SU = 0, 1, 2
NMATS = 3


def _build_mats() -> np.ndarray:
    """[128, 3, 128] fp32, mats[k, m, n] = M_m[k, n] (lhsT layout:
    matmul computes out[mm, n] = sum_k lhsT[k, mm] * rhs[k, n]).
    M_SD: out[m] = in[m-1]; M_SU: out[m] = in[m+1]; edges get 0."""
    eye = np.eye(P, dtype=np.float32)
    sd = np.eye(P, k=1, dtype=np.float32)   # lhsT[k, k+1]=1 -> out[m]=in[m-1]
    su = np.eye(P, k=-1, dtype=np.float32)  # lhsT[k, k-1]=1 -> out[m]=in[m+1]
    m = np.stack([eye, sd, su], axis=0)         # [3, 128, 128]
    return np.ascontiguousarray(m.transpose(1, 0, 2))  # [128, 3, 128]


def build_bass(pv: np.ndarray, ev: np.ndarray) -> bass.Bass:
    # Bacc (not plain Bass): its compile() pass splits multi-semaphore
    # waits into event-semaphore instructions — TRN2 engine instructions
    # support only one sync wait each.
    nc = bacc.Bacc(None)
    world = nc.dram_tensor("world", [S, C, H, W], F32, kind="ExternalInput")
    rand = nc.dram_tensor("rand", [S, H, W], F32, kind="ExternalInput")
    mats = nc.dram_tensor("mats", [P, NMATS, P], F32, kind="ExternalInput")
    out = nc.dram_tensor("out", [S, C, H, W], F32, kind="ExternalOutput")

    with tile.TileContext(nc) as tc:
        with (
            tc.tile_pool(name="const", bufs=1) as const_pool,
            tc.tile_pool(name="wg", bufs=4) as wg_pool,
            tc.tile_pool(name="small", bufs=2) as sm_pool,
            tc.tile_pool(name="mask", bufs=2) as mk_pool,
            tc.tile_pool(name="psum_v", bufs=3, space="PSUM") as pv_pool,
        ):
            mt = const_pool.tile([P, NMATS * P], F32)
            nc.sync.dma_start(out=mt[:], in_=mats.rearrange("k m n -> k (m n)"))

            def mat(m):
                return mt[:, m * P:(m + 1) * P]

            def conv_plane(x, v_name):
                """x: [128, 512] SBUF plane AP (parity layout: partition p
                = rows 2p|2p+1) -> [128,512] SBUF tile with the 3x3
                ones-conv (SAME). Vertical sums in ascending row order:
                v[r] = (x[r-1] + x[r]) + x[r+1]."""
                v = pv_pool.tile([P, PL], F32, name=f"v_{v_name}", tag="v")
                x0, x1 = x[:, 0:BLK], x[:, BLK:PL]   # even rows | odd rows
                # v_even[p] = x1[p-1] + x0[p] + x1[p]
                nc.tensor.matmul(v[:, 0:BLK], mat(M_SD), x1, start=True, stop=False)
                nc.tensor.matmul(v[:, 0:BLK], mat(M_I), x0, start=False, stop=False)
                nc.tensor.matmul(v[:, 0:BLK], mat(M_I), x1, start=False, stop=True)
                # v_odd[p] = x0[p] + x1[p] + x0[p+1]
                nc.tensor.matmul(v[:, BLK:PL], mat(M_I), x0, start=True, stop=False)
                nc.tensor.matmul(v[:, BLK:PL], mat(M_I), x1, start=False, stop=False)
                nc.tensor.matmul(v[:, BLK:PL], mat(M_SU), x0, start=False, stop=True)
                vc = sm_pool.tile([P, PL], F32, name=f"vc_{v_name}", tag=f"vc_{v_name[0]}")
                nc.scalar.copy(vc[:], v[:])
                # horizontal pass (DVE): h_j = (v_{j-1} + v_j) + v_{j+1}
                h = sm_pool.tile([P, PL], F32, name=f"h_{v_name}", tag=f"h_{v_name[0]}")
                for b0 in (0, BLK):
                    s = sm_pool.tile([P, BLK - 1], F32, name=f"s_{v_name}{b0}", tag="s")
                    nc.vector.tensor_add(s[:], vc[:, b0:b0 + BLK - 1], vc[:, b0 + 1:b0 + BLK])
                    nc.vector.tensor_add(
                        h[:, b0 + 1:b0 + BLK - 1], s[:, 0:BLK - 2], vc[:, b0 + 2:b0 + BLK])
                    nc.scalar.copy(h[:, b0:b0 + 1], s[:, 0:1])
                    nc.scalar.copy(h[:, b0 + BLK - 1:b0 + BLK], s[:, BLK - 2:BLK - 1])
                return h

            for s in range(S):
                # ---- loads (one DMA per 10-channel half: 2.5 MiB each) ----
                rt = sm_pool.tile([P, PL], F32, name="rt", tag="rt")
                nc.sync.dma_start(
                    out=rt[:].rearrange("p (q w) -> p q w", w=W),
                    in_=rand[s].rearrange("(p q) w -> p q w", p=P))
                wg = []
                for gi in range(2):
                    g_t = wg_pool.tile([P, HC * PL], F32, name=f"wg{gi}", tag="wg")
                    nc.sync.dma_start(
                        out=g_t[:].rearrange("p (c q w) -> p c q w", w=W, q=2),
                        in_=world[s, gi * HC:(gi + 1) * HC].rearrange(
                            "c (p q) w -> p c q w", p=P))
                    wg.append(g_t)

                def ch(c):
                    return wg[c // HC][:, (c % HC) * PL:((c % HC) + 1) * PL]

                # ---- convolutions ----
                wi = sm_pool.tile([P, PL], F32, name="wi", tag="wi")
                nc.gpsimd.tensor_add(wi[:], ch(ICE), ch(WOOD))
                pc = conv_plane(ch(PLANT), f"pc{s}")
                wic = conv_plane(wi[:], f"wic{s}")

                # ---- comparisons ----
                # 0/1 mask values are exact in bf16 and tt ops run 2x
                def cmp(eng, name, src, op, thr):
                    t = mk_pool.tile([P, PL], BF16, name=name, tag=name, bufs=1)
                    eng.tensor_scalar(
                        out=t[:], in0=src, scalar1=thr, scalar2=None, op0=op)
                    return t

                lt, gt, ge, le = (mybir.AluOpType.is_lt, mybir.AluOpType.is_gt,
                                  mybir.AluOpType.is_ge, mybir.AluOpType.is_le)
                g_m = cmp(nc.gpsimd, "g", ch(WATER), gt, 0.5)
                q05 = cmp(nc.gpsimd, "q05", rt[:], lt, 0.05)
                q2 = cmp(nc.gpsimd, "q2", rt[:], lt, 0.2)
                e_m = cmp(nc.gpsimd, "e", ch(EMPTY), gt, 0.5)
                ge1 = cmp(nc.gpsimd, "ge1", pc[:], ge, 1.0)
                le3 = cmp(nc.gpsimd, "le3", pc[:], le, 3.0)
                gt3 = cmp(nc.gpsimd, "gt3", pc[:], gt, 3.0)
                gt0 = cmp(nc.gpsimd, "gt0", pc[:], gt, 0.0)
                wgt0 = cmp(nc.gpsimd, "wgt0", wic[:], gt, 0.0)

                # ---- mask logic ----
                def tt(eng, name, in0, in1, op, dtype=BF16, bufs=1):
                    t = mk_pool.tile([P, PL], dtype, name=name, tag=name, bufs=bufs)
                    eng.tensor_tensor(t[:], in0, in1, op)
                    return t

                mul, mx = mybir.AluOpType.mult, mybir.AluOpType.max
                dp = tt(nc.gpsimd, "dp", g_m[:], q05[:], mul)
                b_m = tt(nc.vector, "b_m", dp[:], gt3[:], mul, F32, 2)
                a1a = tt(nc.gpsimd, "a1a", dp[:], ge1[:], mul)
                a1 = tt(nc.gpsimd, "a1", a1a[:], le3[:], mul)
                t2a = tt(nc.gpsimd, "t2a", wgt0[:], q2[:], mul)
                t2b = tt(nc.gpsimd, "t2b", t2a[:], e_m[:], mul)
                t2c = tt(nc.gpsimd, "t2c", t2b[:], gt0[:], mul)
                a_m = tt(nc.vector, "a_m", a1[:], t2c[:], mx, F32, 2)
                # copy_predicated requires an integer mask dtype; the mask
                # is duplicated side-by-side so one copy_predicated can
                # blend a pair of adjacent channels
                ab = mk_pool.tile([P, 2 * PL], mybir.dt.uint8, name="ab",
                                  tag="ab", bufs=2)
                nc.vector.tensor_tensor(ab[:, 0:PL], a_m[:], b_m[:], mx)
                nc.gpsimd.tensor_copy(ab[:, PL:2 * PL], ab[:, 0:PL])

                # ---- per-channel blend + stores ----
                # q = a*pv[c] (ACT) then q += b*ev[c] (GPSIMD fused) —
                # exact: every selected pixel gets pv, ev, or fl(pv+ev).
                # channel pairs share one [128, 1024] q tile so a single
                # copy_predicated blends two channels per DVE op
                for gi in range(2):
                    for ci in range(0, HC, 2):
                        pr = (gi * HC + ci) // 2
                        qp = mk_pool.tile([P, 2 * PL], F32, name=f"qp_{pr}",
                                          tag=f"qp{pr % 3}", bufs=2)
                        for k in range(2):
                            c = gi * HC + ci + k
                            q1 = mk_pool.tile([P, PL], F32, name=f"q1_{c}",
                                              tag=f"q1{c % 3}", bufs=2)
                            nc.scalar.mul(q1[:], a_m[:], float(np.float32(pv[c])))
                            qs = qp[:, k * PL:(k + 1) * PL]
                            if k == 0:
                                # fused b*ev + q1 on DVE (Pool lacks this op)
                                nc.vector.scalar_tensor_tensor(
                                    out=qs, in0=b_m[:],
                                    scalar=float(np.float32(ev[c])), in1=q1[:],
                                    op0=mul, op1=mybir.AluOpType.add)
                            else:
                                qb = mk_pool.tile([P, PL], F32, name=f"qb_{c}",
                                                  tag=f"qb{c % 3}", bufs=2)
                                nc.gpsimd.tensor_scalar(
                                    out=qb[:], in0=b_m[:],
                                    scalar1=float(np.float32(ev[c])),
                                    scalar2=None, op0=mul)
                                nc.gpsimd.tensor_add(qs, qb[:], q1[:])
                        nc.vector.copy_predicated(
                            wg[gi][:, ci * PL:(ci + 2) * PL], ab[:], qp[:])
                    nc.sync.dma_start(
                        out=out[s, gi * HC:(gi + 1) * HC].rearrange(
                            "c (p q) w -> p c q w", p=P),
                        in_=wg[gi][:].rearrange("p (c q w) -> p c q w", w=W, q=2))
    nc.compile()
    return nc


_NC_CACHE = {}


def _get_nc(pv_key, pv, ev):
    if pv_key not in _NC_CACHE:
        _NC_CACHE[pv_key] = build_bass(pv, ev)
    return _NC_CACHE[pv_key]


def kernel(**inputs: np.ndarray) -> np.ndarray:
    world = np.ascontiguousarray(np.asarray(inputs["world"], dtype=np.float32))
    rand = np.ascontiguousarray(
        np.asarray(inputs["rand_interact"], dtype=np.float32)[:, 0])
    pv = np.asarray(inputs["elem_vec_plant"], dtype=np.float32).reshape(-1)
    ev = np.asarray(inputs["elem_vec_empty"], dtype=np.float32).reshape(-1)
    mats = _build_mats()

    nc = _get_nc((pv.tobytes(), ev.tobytes()), pv, ev)
    in_maps = [
        {
            "world": world[i * S:(i + 1) * S],
            "rand": rand[i * S:(i + 1) * S],
            "mats": mats,
        }
        for i in range(N_CORES)
    ]
    res = run_bass_kernel_spmd(nc, in_maps, list(range(N_CORES)))
    return np.concatenate([res.results[i]["out"] for i in range(N_CORES)], axis=0)
